# revision 1
# baseline (speedup 1.0000x reference)
"""Causal self-attention (B=4, T=2048, C=1024, 16 heads) on 8 NeuronCores.

Sharding: core i handles batch b = i//2 and head-group hg = i%2 (8 heads each).
Each core computes the qkv projection for its heads, causal attention, and a
partial output projection (contraction over its 512 channels). The host sums
the two partial outputs per batch and adds b_proj.

Device kernel design (matmuls in bf16 with fp32 PSUM accumulation):
  - x^T (augmented with a ones row) resident in SBUF; q,k are produced
    transposed ([head_dim, T] per head, two heads stacked per 128-partition
    tile) so score blocks S^T = k^T.T @ q^T need no transposes. The two heads
    of a tile run concurrently in the PE array via row tiling (K=64 at base
    partitions 0 and 64).
  - softmax along the free axis without max subtraction (scores are small:
    |s|/8 <= ~0.5): exp on ScalarE straight out of PSUM; the causal triangle
    is masked by a 0/1 multiply on diagonal blocks only, and fully-masked
    column ranges are never computed.
  - v is produced naturally ([T, head_dim]) with an extra all-ones column per
    head (baked into the weight matrix), so P^T.T @ v_aug also yields the
    softmax denominator Z per row for free.
  - y stays unnormalized until just before the projection: 1/Z (fast DVE
    reciprocal) is broadcast across each head's 64 rows with a tiny
    outer-product matmul and applied in one tensor_tensor multiply.
  - Emission interleaving: the attention phase is ScalarE(exp)-bound, the
    projections are PE-bound. Projection work is chopped into small "filler"
    units that are emitted between attention j-steps so the Tile scheduler
    places them in the PE stream's dependency stalls.
"""

import numpy as np
import ml_dtypes

B, T, C = 4, 2048, 1024
NH, HS = 16, 64
NCORES = 8
LH = 8          # local heads per core
BF16 = ml_dtypes.bfloat16

_nc_cache = {}


def _build_nc():
    from collections import deque

    import concourse.bacc as bacc
    import concourse.tile as tile
    from concourse import mybir

    bf = mybir.dt.bfloat16
    f32 = mybir.dt.float32
    Exp = mybir.ActivationFunctionType.Exp

    nc = bacc.Bacc("TRN2", target_bir_lowering=False, debug=False, num_devices=NCORES)

    xT = nc.dram_tensor("xT", [C + 1, T], bf, kind="ExternalInput")
    wqk = nc.dram_tensor("wqk", [C, 1024], bf, kind="ExternalInput")
    wqkb2 = nc.dram_tensor("wqkb2", [128, 8], f32, kind="ExternalInput")
    wv = nc.dram_tensor("wv", [C + 1, 520], bf, kind="ExternalInput")
    wp = nc.dram_tensor("wp", [512, 1024], bf, kind="ExternalInput")
    tri = nc.dram_tensor("tri", [128, 128], bf, kind="ExternalInput")
    esel = nc.dram_tensor("esel", [1, 256], f32, kind="ExternalInput")
    out = nc.dram_tensor("out", [T, C], f32, kind="ExternalOutput")

    with tile.TileContext(nc) as tc:
        with (
            tc.tile_pool(name="consts", bufs=1) as consts,
            tc.tile_pool(name="spool", bufs=4, space="PSUM") as spool,
            tc.tile_pool(name="ypool", bufs=2, space="PSUM") as ypool,
            tc.tile_pool(name="mpool", bufs=2, space="PSUM") as mpool,
            tc.tile_pool(name="ppool", bufs=6) as ppool,
            tc.tile_pool(name="opool", bufs=3) as opool,
            tc.tile_pool(name="rpool", bufs=11) as rpool,
        ):
            xT_sb = consts.tile([128, 8 * T], bf)
            xones = consts.tile([1, T], bf)
            wqk_sb = consts.tile([128, 8 * 1024], bf)
            wqkb2_sb = consts.tile([128, 8], f32)
            wv_sb = consts.tile([128, 8 * 520], bf)
            wvb = consts.tile([1, 520], bf)
            wp_sb = consts.tile([128, 4 * 1024], bf)
            tri_sb = consts.tile([128, 128], bf)
            esel_sb = consts.tile([1, 256], f32)
            qk_sb = consts.tile([128, 8 * T], bf)
            v_sb = consts.tile([128, 16 * 520], bf)
            y_sb = [consts.tile([128, T], bf, name=f"y_sb{g}") for g in range(4)]
            yp_sb = [consts.tile([128, T], bf, name=f"yp_sb{g}") for g in range(4)]

            # ---- input DMAs, ordered so phase B can start early:
            # wv + x columns 0-511 first, then the rest.
            for kb in range(8):
                nc.sync.dma_start(out=wv_sb[:, 520 * kb : 520 * kb + 520],
                                  in_=wv[128 * kb : 128 * kb + 128, :])
            nc.sync.dma_start(out=wvb[0:1, :], in_=wv[1024:1025, :])
            nc.sync.dma_start(out=xones[0:1, :], in_=xT[1024:1025, :])
            for kb in range(8):
                nc.sync.dma_start(out=xT_sb[:, 2048 * kb : 2048 * kb + 512],
                                  in_=xT[128 * kb : 128 * kb + 128, 0:512])
            for kb in range(8):
                nc.sync.dma_start(out=wqk_sb[:, 1024 * kb : 1024 * kb + 1024],
                                  in_=wqk[128 * kb : 128 * kb + 128, :])
            nc.sync.dma_start(out=wqkb2_sb[:, :], in_=wqkb2[:, :])
            nc.sync.dma_start(out=tri_sb[:, :], in_=tri[:, :])
            nc.sync.dma_start(out=esel_sb[:, :], in_=esel[:, :])
            for tq in range(1, 4):
                for kb in range(8):
                    nc.sync.dma_start(
                        out=xT_sb[:, 2048 * kb + 512 * tq : 2048 * kb + 512 * tq + 512],
                        in_=xT[128 * kb : 128 * kb + 128, 512 * tq : 512 * tq + 512])
            for kb in range(4):
                nc.sync.dma_start(out=wp_sb[:, 1024 * kb : 1024 * kb + 1024],
                                  in_=wp[128 * kb : 128 * kb + 128, :])

            # ---- emission units -------------------------------------------
            def v_group(t, half):
                """v projection for T-tile t, heads half*4..half*4+3 (260 cols)."""
                units = []
                ps_box = []

                def mk_mm(kb):
                    def f():
                        if kb == 0:
                            ps_box.append(mpool.tile([128, 260], f32, tag="mm", name="vps"))
                        ps = ps_box[0]
                        if kb < 8:
                            lhsT = xT_sb[:, 2048 * kb + 128 * t : 2048 * kb + 128 * t + 128]
                            rhs = wv_sb[:, 520 * kb + 260 * half : 520 * kb + 260 * half + 260]
                        else:
                            lhsT = xones[0:1, 128 * t : 128 * t + 128]
                            rhs = wvb[0:1, 260 * half : 260 * half + 260]
                        nc.tensor.matmul(ps[:, :], lhsT, rhs, start=(kb == 0), stop=(kb == 8))
                    return f

                for kb in range(9):
                    units.append(mk_mm(kb))

                def epi():
                    nc.vector.tensor_copy(
                        v_sb[:, 520 * t + 260 * half : 520 * t + 260 * half + 260],
                        ps_box[0][:, :])
                units.append(epi)
                return units

            def qk_group(o, c):
                """q/k projection chunk: o-tile o (0-3 q, 4-7 k), T-chunk c."""
                units = []
                ps_box = []

                def mk_mm(kb):
                    def f():
                        if kb == 0:
                            ps_box.append(mpool.tile([128, 512], f32, tag="mm", name="qkps"))
                        ps = ps_box[0]
                        nc.tensor.matmul(
                            ps[:, :],
                            wqk_sb[:, 1024 * kb + 128 * o : 1024 * kb + 128 * o + 128],
                            xT_sb[:, 2048 * kb + 512 * c : 2048 * kb + 512 * c + 512],
                            start=(kb == 0), stop=(kb == 7))
                    return f

                for kb in range(8):
                    units.append(mk_mm(kb))

                def epi():
                    nc.vector.tensor_scalar_add(
                        qk_sb[:, 2048 * o + 512 * c : 2048 * o + 512 * c + 512],
                        ps_box[0][:, :],
                        wqkb2_sb[:, o : o + 1])
                units.append(epi)
                return units

            # ---- preamble: v tiles 0-3 (heads 0-3), qk chunk 0 for pair 0
            for t in range(4):
                for u in v_group(t, 0):
                    u()
            for o in (0, 4):
                for u in qk_group(o, 0):
                    u()

            # ---- filler queue: remaining projections in data-dependency
            # order; emitted piecewise between attention steps. ensure()
            # force-drains the queue up to a required producer.
            fillers = deque()
            emitted = set()
            emitted.update({("v", t, 0) for t in range(4)})
            emitted.add(("qk", 0, 0))
            emitted.add(("qk", 4, 0))

            def q_v(t, half):
                fillers.append((("v", t, half), v_group(t, half)))

            def q_qk(o, c):
                fillers.append((("qk", o, c), qk_group(o, c)))

            # g=0 tail chunks + v(.,0) just ahead of their use
            for c in range(1, 4):
                q_qk(0, c)
                q_qk(4, c)
                for t in range(4 * c, 4 * c + 4):
                    q_v(t, 0)
            # g=1
            for c in range(4):
                q_qk(1, c)
                q_qk(5, c)
            # g=2 (+ v half 1)
            for c in range(4):
                q_qk(2, c)
                q_qk(6, c)
                for t in range(4 * c, 4 * c + 4):
                    q_v(t, 1)
            # g=3
            for c in range(4):
                q_qk(3, c)
                q_qk(7, c)

            pending = deque()  # units of the partially-emitted current group
            inflight = [None]   # key of the group in `pending`

            def _step():
                pending.popleft()()
                if not pending:
                    emitted.add(inflight[0])
                    inflight[0] = None

            def pump(n):
                for _ in range(n):
                    if pending:
                        _step()
                    elif fillers:
                        key, units = fillers.popleft()
                        inflight[0] = key
                        pending.extend(units)
                        _step()

            def ensure(key):
                if key in emitted:
                    return
                while pending:
                    _step()
                while key not in emitted:
                    k2, units = fillers.popleft()
                    inflight[0] = k2
                    pending.extend(units)
                    while pending:
                        _step()

            # ---- attention ------------------------------------------------
            for g in range(4):
                rs_g = {}
                for c in range(4):
                    ensure(("qk", g, c))
                    ensure(("qk", 4 + g, c))
                    y01 = [ypool.tile([65, 512], f32, tag="y", name=f"yaug{h}") for h in range(2)]
                    nj = 4 * c + 4
                    for j in range(nj):
                        ensure(("qk", 4 + g, j // 4))
                        ensure(("v", j, g // 2))
                        d = max(0, j - 4 * c)
                        w = 512 - 128 * d
                        koff = 2048 * (4 + g) + 128 * j
                        qoff = 2048 * g + 512 * c + 128 * d
                        Ss, pTs = [], []
                        for h in range(2):
                            lo, hi = 64 * h, 64 * h + 64
                            S = spool.tile([128, 512], f32, tag="s", name="S")
                            nc.tensor.matmul(S[:, 0:w],
                                             qk_sb[lo:hi, koff : koff + 128],
                                             qk_sb[lo:hi, qoff : qoff + w],
                                             start=True, stop=True)
                            Ss.append(S)
                        for h in range(2):
                            pT = ppool.tile([128, 512], bf, tag="p", name="pT")
                            nc.scalar.activation(pT[:, 128 * d : 512], Ss[h][:, 0:w], Exp, scale=0.125)
                            if j >= 4 * c:
                                nc.vector.tensor_mul(pT[:, 128 * d : 128 * d + 128],
                                                     pT[:, 128 * d : 128 * d + 128],
                                                     tri_sb[:, :])
                            pTs.append(pT)
                        for h in range(2):
                            nc.tensor.matmul(
                                y01[h][:, 128 * d : 512],
                                v_sb[:, 520 * j + 65 * (2 * g + h) : 520 * j + 65 * (2 * g + h) + 65],
                                pTs[h][:, 128 * d : 512],
                                start=(j == 0), stop=(j == nj - 1))
                        pump(2)

                    # epilogue: stash y, extract Z, 1/Z, broadcast, normalize
                    for h in range(2):
                        zc = rpool.tile([1, 512], f32, tag="r", name="zc")
                        nc.vector.tensor_copy(zc[0:1, :], y01[h][64:65, :])
                        r = rpool.tile([1, 512], f32, tag="r", name="rr")
                        nc.vector.reciprocal_approx_fast(r[0:1, :], zc[0:1, :])
                        rs_g[(c, h)] = r
                        nc.vector.tensor_copy(
                            y_sb[g][64 * h : 64 * h + 64, 512 * c : 512 * c + 512],
                            y01[h][0:64, :])
                    pump(6)

                # deferred normalization: recips are complete by now, so the
                # broadcast matmuls never stall the PE on the DVE chain
                for c2 in range(4):
                    pb = mpool.tile([128, 512], f32, tag="mm", name="pb")
                    for h in range(2):
                        nc.tensor.matmul(pb[:, :],
                                         esel_sb[0:1, 128 * h : 128 * h + 128],
                                         rs_g[(c2, h)][0:1, :],
                                         start=(h == 0), stop=(h == 1))
                    nc.vector.tensor_mul(yp_sb[g][:, 512 * c2 : 512 * c2 + 512],
                                         y_sb[g][:, 512 * c2 : 512 * c2 + 512],
                                         pb[:, :])

            while pending or fillers:
                pump(1)

            # ---- output projection (partial over this core's channels)
            for t in range(16):
                for o2 in range(2):
                    ps = mpool.tile([128, 512], f32, tag="mm", name="fps")
                    for g in range(4):
                        nc.tensor.matmul(ps[:, :],
                                         yp_sb[g][:, 128 * t : 128 * t + 128],
                                         wp_sb[:, 1024 * g + 512 * o2 : 1024 * g + 512 * o2 + 512],
                                         start=(g == 0), stop=(g == 3))
                    ot = opool.tile([128, 512], f32, tag="o", name="ot")
                    nc.vector.tensor_copy(ot[:, :], ps[:, :])
                    nc.sync.dma_start(out=out[128 * t : 128 * t + 128, 512 * o2 : 512 * o2 + 512],
                                      in_=ot[:, :])

    nc.compile()
    return nc


def _host_inputs(x, W_qkv, b_qkv):
    """Per-core input tensors (host-side slicing/transpose/augment/cast)."""
    xTs = []
    for b in range(B):
        xa = np.empty((C + 1, T), dtype=BF16)
        xa[:C] = x[b].T.astype(BF16)
        xa[C] = np.ones(T, dtype=BF16)
        xTs.append(xa)

    wqks, wqkb2s, wvs = [], [], []
    for hg in range(2):
        qs, ks, vs = 512 * hg, 1024 + 512 * hg, 2048 + 512 * hg
        wqkm = np.empty((C, 1024), dtype=BF16)
        wqkm[:, 0:512] = W_qkv[qs : qs + 512].T.astype(BF16)
        wqkm[:, 512:1024] = W_qkv[ks : ks + 512].T.astype(BF16)
        wqks.append(wqkm)
        bqk = np.concatenate([b_qkv[qs : qs + 512], b_qkv[ks : ks + 512]])
        wqkb2s.append(np.ascontiguousarray(bqk.reshape(8, 128).T).astype(np.float32))

        wva = np.zeros((C + 1, 520), dtype=BF16)
        Wv = W_qkv[vs : vs + 512]
        bv = b_qkv[vs : vs + 512]
        for h in range(LH):
            wva[:C, 65 * h : 65 * h + 64] = Wv[64 * h : 64 * h + 64].T.astype(BF16)
            wva[C, 65 * h : 65 * h + 64] = bv[64 * h : 64 * h + 64].astype(BF16)
            wva[C, 65 * h + 64] = 1.0
        wvs.append(wva)

    tri = np.triu(np.ones((128, 128))).astype(BF16)
    esel = np.zeros((1, 256), dtype=np.float32)
    esel[0, 0:64] = 1.0        # head-even outer product -> rows 0-63
    esel[0, 192:256] = 1.0     # head-odd  outer product -> rows 64-127
    return xTs, wqks, wqkb2s, wvs, tri, esel


def kernel(x, W_qkv, b_qkv, W_proj, b_proj):
    from concourse.bass_utils import run_bass_kernel_spmd

    if "nc" not in _nc_cache:
        _nc_cache["nc"] = _build_nc()
    nc = _nc_cache["nc"]

    x = np.asarray(x, dtype=np.float32)
    W_qkv = np.asarray(W_qkv, dtype=np.float32)
    b_qkv = np.asarray(b_qkv, dtype=np.float32)
    W_proj = np.asarray(W_proj, dtype=np.float32)
    b_proj = np.asarray(b_proj, dtype=np.float32)

    xTs, wqks, wqkb2s, wvs, tri, esel = _host_inputs(x, W_qkv, b_qkv)
    wps = [np.ascontiguousarray(W_proj[:, 512 * hg : 512 * hg + 512].T).astype(BF16)
           for hg in range(2)]

    in_maps = []
    for core in range(NCORES):
        b, hg = core // 2, core % 2
        in_maps.append({
            "xT": xTs[b],
            "wqk": wqks[hg],
            "wqkb2": wqkb2s[hg],
            "wv": wvs[hg],
            "wp": wps[hg],
            "tri": tri,
            "esel": esel,
        })

    try:
        res = run_bass_kernel_spmd(nc, in_maps, core_ids=list(range(NCORES)),
                                   **_nc_cache.get("run_kwargs", {}))
    except Exception:
        # transient axon-terminal device errors recover on retry
        import time as _time
        _time.sleep(2.0)
        res = run_bass_kernel_spmd(nc, in_maps, core_ids=list(range(NCORES)),
                                   **_nc_cache.get("run_kwargs", {}))
    _nc_cache["last_result"] = res

    out = np.empty((B, T, C), dtype=np.float32)
    for b in range(B):
        out[b] = res.results[2 * b]["out"] + res.results[2 * b + 1]["out"] + b_proj
    return out



# revision 2
# speedup vs baseline: 1.3043x; 1.3043x over previous
"""Causal self-attention (B=4, T=2048, C=1024, 16 heads) on 8 NeuronCores.

Sharding: core i handles batch b = i//2 and head-group hg = i%2 (8 heads each).
Each core computes the qkv projection for its heads, causal attention, and a
partial output projection (contraction over its 512 channels). The host sums
the two partial outputs per batch and adds b_proj + W_proj @ b_v (the v-bias
is never added on device: softmax rows sum to 1, so its effect on y is the
constant +b_v, which commutes with the projection).

Device kernel design (matmuls in bf16 with fp32 PSUM accumulation):
  - x^T resident in SBUF (packed on host so DMAs are contiguous); q,k are
    produced transposed ([head_dim, T] per head, two heads stacked per
    128-partition tile) so score blocks S^T = k^T.T @ q^T need no transposes.
  - softmax along the free axis without max subtraction (scores are small:
    |s|/8 <= ~0.5): one exp on ScalarE per j-step covering both heads (the
    two heads' score blocks live in one 2-bank PSUM tile); the causal
    triangle is masked by a 0/1 multiply on diagonal blocks only, and
    fully-masked column ranges are never computed.
  - v is produced naturally ([T, head_dim]); each head's PV lhsT is a
    128-column block [ones(64) | v(64)], so the PV matmul's output rows 0-63
    are the softmax denominator Z broadcast 64-fold -- same stream length as
    a 64-row output, i.e. the Z broadcast is free on the PE. The epilogue is
    then just reciprocal_approx_fast on rows 0-63 and one tensor multiply
    against rows 64-127, writing the normalized y straight to SBUF in bf16.
  - Emission interleaving: the attention phase is roughly balanced between
    ScalarE(exp) and the PE; projection work is chopped into small "filler"
    units emitted between attention j-steps so the Tile scheduler places
    them in the PE stream's dependency stalls.
  - Output is written bf16 in small per-queue chunks to cut the final DMA
    tail; the host accumulates the two partials in fp32.
"""

import numpy as np
import ml_dtypes

B, T, C = 4, 2048, 1024
NH, HS = 16, 64
NCORES = 8
LH = 8          # local heads per core
BF16 = ml_dtypes.bfloat16

_nc_cache = {}


def _build_nc():
    from collections import deque

    import concourse.bacc as bacc
    import concourse.tile as tile
    from concourse import mybir

    bf = mybir.dt.bfloat16
    f32 = mybir.dt.float32
    Exp = mybir.ActivationFunctionType.Exp

    nc = bacc.Bacc("TRN2", target_bir_lowering=False, debug=False, num_devices=NCORES)

    # packed layouts (host side): see _host_inputs
    xp = nc.dram_tensor("xp", [128, 16384], bf, kind="ExternalInput")
    wvp = nc.dram_tensor("wvp", [128, 4096], bf, kind="ExternalInput")
    wqkp = nc.dram_tensor("wqkp", [128, 8192], bf, kind="ExternalInput")
    wqkb2 = nc.dram_tensor("wqkb2", [128, 8], f32, kind="ExternalInput")
    wpp = nc.dram_tensor("wpp", [128, 4096], bf, kind="ExternalInput")
    tri = nc.dram_tensor("tri", [128, 128], bf, kind="ExternalInput")
    out = nc.dram_tensor("out", [T, C], bf, kind="ExternalOutput")

    with tile.TileContext(nc) as tc:
        with (
            tc.tile_pool(name="consts", bufs=1) as consts,
            tc.tile_pool(name="spool", bufs=2, space="PSUM") as spool,
            tc.tile_pool(name="ypool", bufs=2, space="PSUM") as ypool,
            tc.tile_pool(name="mpool", bufs=2, space="PSUM") as mpool,
            tc.tile_pool(name="ppool", bufs=4) as ppool,
            tc.tile_pool(name="opool", bufs=3) as opool,
            tc.tile_pool(name="rpool", bufs=4) as rpool,
        ):
            # x^T columns grouped quarter-major: col = 4096*q + 512*kb + tc
            xT_sb = consts.tile([128, 16384], bf)
            wv_sb = consts.tile([128, 4096], bf)
            wqk_sb = consts.tile([128, 8192], bf)
            wqkb2_sb = consts.tile([128, 8], f32)
            wp_sb = consts.tile([128, 4096], bf)
            tri_sb = consts.tile([128, 128], bf)
            qk_sb = consts.tile([128, 16384], bf)
            # per (t, head): 128 cols = [ones(64) | v(64)]
            v_sb = consts.tile([128, 16, 8, 128], bf)
            yp_sb = [consts.tile([128, T], bf, name=f"yp_sb{g}") for g in range(4)]

            # ones columns for the Z-broadcast rows, off the critical engines
            for t in range(16):
                nc.gpsimd.memset(v_sb[:, t, :, 0:64], 1.0)

            # ---- input DMAs, ordered so the v/qk preamble can start early
            for kb in range(8):
                nc.sync.dma_start(out=wv_sb[:, 512 * kb : 512 * kb + 512],
                                  in_=wvp[:, 512 * kb : 512 * kb + 512])
            for kb in range(8):
                nc.sync.dma_start(out=xT_sb[:, 512 * kb : 512 * kb + 512],
                                  in_=xp[:, 512 * kb : 512 * kb + 512])
            for i in range(16):
                nc.sync.dma_start(out=wqk_sb[:, 512 * i : 512 * i + 512],
                                  in_=wqkp[:, 512 * i : 512 * i + 512])
            nc.sync.dma_start(out=wqkb2_sb[:, :], in_=wqkb2[:, :])
            nc.sync.dma_start(out=tri_sb[:, :], in_=tri[:, :])
            for i in range(12):
                nc.sync.dma_start(out=xT_sb[:, 4096 + 1024 * i : 4096 + 1024 * i + 1024],
                                  in_=xp[:, 4096 + 1024 * i : 4096 + 1024 * i + 1024])
            for i in range(4):
                nc.sync.dma_start(out=wp_sb[:, 1024 * i : 1024 * i + 1024],
                                  in_=wpp[:, 1024 * i : 1024 * i + 1024])

            def xslice(kb, t):
                """x^T block for contraction chunk kb, T-tile t (128 cols)."""
                o = 4096 * (t // 4) + 512 * kb + 128 * (t % 4)
                return xT_sb[:, o : o + 128]

            # ---- emission units -------------------------------------------
            def v_group(t):
                """v projection for T-tile t, all 8 heads (512 cols)."""
                units = []
                ps_box = []

                def mk_mm(kb):
                    def f():
                        if kb == 0:
                            ps_box.append(mpool.tile([128, 8, 64], f32, tag="mm", name="vps"))
                        nc.tensor.matmul(ps_box[0][:, :, :], xslice(kb, t),
                                         wv_sb[:, 512 * kb : 512 * kb + 512],
                                         start=(kb == 0), stop=(kb == 7))
                    return f

                for kb in range(8):
                    units.append(mk_mm(kb))

                def epi():
                    nc.vector.tensor_copy(v_sb[:, t, :, 64:128], ps_box[0][:, :, :])
                units.append(epi)
                return units

            def qk_group(o, c):
                """q/k projection chunk: o-tile o (0-3 q, 4-7 k), T-chunk c."""
                units = []
                ps_box = []

                def mk_mm(kb):
                    def f():
                        if kb == 0:
                            ps_box.append(mpool.tile([128, 512], f32, tag="mm", name="qkps"))
                        nc.tensor.matmul(
                            ps_box[0][:, :],
                            wqk_sb[:, 1024 * kb + 128 * o : 1024 * kb + 128 * o + 128],
                            xT_sb[:, 4096 * c + 512 * kb : 4096 * c + 512 * kb + 512],
                            start=(kb == 0), stop=(kb == 7))
                    return f

                for kb in range(8):
                    units.append(mk_mm(kb))

                def epi():
                    nc.vector.tensor_scalar_add(
                        qk_sb[:, 2048 * o + 512 * c : 2048 * o + 512 * c + 512],
                        ps_box[0][:, :],
                        wqkb2_sb[:, o : o + 1])
                units.append(epi)
                return units

            # ---- preamble: v tiles 0-3, qk chunk 0 for pair 0
            for o in (0, 4):
                for u in qk_group(o, 0):
                    u()
            for t in range(4):
                for u in v_group(t):
                    u()

            # ---- filler queue: remaining projections in data-dependency
            # order; emitted piecewise between attention steps. ensure()
            # force-drains the queue up to a required producer.
            fillers = deque()
            emitted = set()
            emitted.update({("v", t) for t in range(4)})
            emitted.add(("qk", 0, 0))
            emitted.add(("qk", 4, 0))

            def q_v(t):
                fillers.append((("v", t), v_group(t)))

            def q_qk(o, c):
                fillers.append((("qk", o, c), qk_group(o, c)))

            # g=0 tail chunks + v(.) just ahead of their use
            for c in range(1, 4):
                q_qk(0, c)
                q_qk(4, c)
                for t in range(4 * c, 4 * c + 4):
                    q_v(t)
            # g=1..3
            for g in range(1, 4):
                for c in range(4):
                    q_qk(g, c)
                    q_qk(4 + g, c)

            pending = deque()  # units of the partially-emitted current group
            inflight = [None]   # key of the group in `pending`

            def _step():
                pending.popleft()()
                if not pending:
                    emitted.add(inflight[0])
                    inflight[0] = None

            def pump(n):
                for _ in range(n):
                    if pending:
                        _step()
                    elif fillers:
                        key, units = fillers.popleft()
                        inflight[0] = key
                        pending.extend(units)
                        _step()

            def ensure(key):
                if key in emitted:
                    return
                while pending:
                    _step()
                while key not in emitted:
                    k2, units = fillers.popleft()
                    inflight[0] = k2
                    pending.extend(units)
                    while pending:
                        _step()

            # ---- attention ------------------------------------------------
            for g in range(4):
                for c in range(4):
                    ensure(("qk", g, c))
                    ensure(("qk", 4 + g, c))
                    y01 = [ypool.tile([128, 512], f32, tag="y", name=f"yzaug{h}") for h in range(2)]
                    nj = 4 * c + 4
                    for j in range(nj):
                        ensure(("qk", 4 + g, j // 4))
                        ensure(("v", j))
                        d = max(0, j - 4 * c)
                        w = 512 - 128 * d
                        koff = 2048 * (4 + g) + 128 * j
                        qoff = 2048 * g + 512 * c + 128 * d
                        S = spool.tile([128, 2, 512], f32, tag="s", name="S")
                        for h in range(2):
                            lo, hi = 64 * h, 64 * h + 64
                            nc.tensor.matmul(S[:, h, 0:w],
                                             qk_sb[lo:hi, koff : koff + 128],
                                             qk_sb[lo:hi, qoff : qoff + w],
                                             start=True, stop=True)
                        pT = ppool.tile([128, 2, 512], bf, tag="p", name="pT")
                        nc.scalar.activation(pT[:, :, 128 * d : 512], S[:, :, 0:w],
                                             Exp, scale=0.125)
                        if j >= 4 * c:
                            for h in range(2):
                                nc.vector.tensor_mul(pT[:, h, 128 * d : 128 * d + 128],
                                                     pT[:, h, 128 * d : 128 * d + 128],
                                                     tri_sb[:, :])
                        for h in range(2):
                            nc.tensor.matmul(
                                y01[h][:, 128 * d : 512],
                                v_sb[:, j, 2 * g + h, :],
                                pT[:, h, 128 * d : 512],
                                start=(j == 0), stop=(j == nj - 1))
                        pump(2)

                    # epilogue: rows 0-63 of y01 hold Z broadcast 64-fold;
                    # reciprocal + one multiply yields normalized y in bf16
                    for h in range(2):
                        rz = rpool.tile([64, 512], f32, tag="r", name="rz")
                        nc.vector.reciprocal_approx_fast(rz[:, :], y01[h][0:64, :])
                        nc.vector.tensor_mul(
                            yp_sb[g][64 * h : 64 * h + 64, 512 * c : 512 * c + 512],
                            y01[h][64:128, :],
                            rz[:, :])
                    pump(6)

            while pending or fillers:
                pump(1)

            # ---- output projection (partial over this core's channels)
            for t in range(16):
                for o2 in range(2):
                    ps = mpool.tile([128, 512], f32, tag="mm", name="fps")
                    for g in range(4):
                        nc.tensor.matmul(ps[:, :],
                                         yp_sb[g][:, 128 * t : 128 * t + 128],
                                         wp_sb[:, 1024 * g + 512 * o2 : 1024 * g + 512 * o2 + 512],
                                         start=(g == 0), stop=(g == 3))
                    ot = opool.tile([128, 512], bf, tag="o", name="ot")
                    nc.vector.tensor_copy(ot[:, :], ps[:, :])
                    for s in range(2):
                        nc.sync.dma_start(
                            out=out[128 * t : 128 * t + 128,
                                    512 * o2 + 256 * s : 512 * o2 + 256 * s + 256],
                            in_=ot[:, 256 * s : 256 * s + 256])

    nc.compile()
    return nc


def _host_inputs(x, W_qkv, b_qkv):
    """Per-core input tensors (host-side slicing/transpose/pack/cast)."""
    xps = []
    for b in range(B):
        xt = np.ascontiguousarray(x[b].T).astype(BF16)          # (C, T)
        # col = 4096*q + 512*kb + tc
        xps.append(np.ascontiguousarray(
            xt.reshape(8, 128, 4, 512).transpose(1, 2, 0, 3).reshape(128, 16384)))

    wqks, wqkb2s, wvs = [], [], []
    for hg in range(2):
        qs, ks, vs = 512 * hg, 1024 + 512 * hg, 2048 + 512 * hg
        wqkm = np.empty((C, 1024), dtype=BF16)
        wqkm[:, 0:512] = W_qkv[qs : qs + 512].T.astype(BF16)
        wqkm[:, 512:1024] = W_qkv[ks : ks + 512].T.astype(BF16)
        wqks.append(np.ascontiguousarray(
            wqkm.reshape(8, 128, 1024).transpose(1, 0, 2).reshape(128, 8192)))
        bqk = np.concatenate([b_qkv[qs : qs + 512], b_qkv[ks : ks + 512]])
        wqkb2s.append(np.ascontiguousarray(bqk.reshape(8, 128).T).astype(np.float32))

        wvm = W_qkv[vs : vs + 512].T.astype(BF16)               # (C, 512)
        wvs.append(np.ascontiguousarray(
            wvm.reshape(8, 128, 512).transpose(1, 0, 2).reshape(128, 4096)))

    tri = np.triu(np.ones((128, 128))).astype(BF16)
    return xps, wqks, wqkb2s, wvs, tri


def kernel(x, W_qkv, b_qkv, W_proj, b_proj):
    from concourse.bass_utils import run_bass_kernel_spmd

    if "nc" not in _nc_cache:
        _nc_cache["nc"] = _build_nc()
    nc = _nc_cache["nc"]

    x = np.asarray(x, dtype=np.float32)
    W_qkv = np.asarray(W_qkv, dtype=np.float32)
    b_qkv = np.asarray(b_qkv, dtype=np.float32)
    W_proj = np.asarray(W_proj, dtype=np.float32)
    b_proj = np.asarray(b_proj, dtype=np.float32)

    xps, wqks, wqkb2s, wvs, tri = _host_inputs(x, W_qkv, b_qkv)
    wps = []
    for hg in range(2):
        wpm = np.ascontiguousarray(W_proj[:, 512 * hg : 512 * hg + 512].T).astype(BF16)
        wps.append(np.ascontiguousarray(
            wpm.reshape(4, 128, 1024).transpose(1, 0, 2).reshape(128, 4096)))

    in_maps = []
    for core in range(NCORES):
        b, hg = core // 2, core % 2
        in_maps.append({
            "xp": xps[b],
            "wqkp": wqks[hg],
            "wqkb2": wqkb2s[hg],
            "wvp": wvs[hg],
            "wpp": wps[hg],
            "tri": tri,
        })

    try:
        res = run_bass_kernel_spmd(nc, in_maps, core_ids=list(range(NCORES)),
                                   **_nc_cache.get("run_kwargs", {}))
    except Exception:
        # transient axon-terminal device errors recover on retry
        import time as _time
        _time.sleep(2.0)
        res = run_bass_kernel_spmd(nc, in_maps, core_ids=list(range(NCORES)),
                                   **_nc_cache.get("run_kwargs", {}))
    _nc_cache["last_result"] = res

    # v-bias never reaches the device: y = P(v+b)/Z = Pv/Z + b_v, so its
    # projected effect is the constant W_proj @ b_v, folded in here.
    bias = b_proj + W_proj @ b_qkv[2 * C : 3 * C]
    out = np.empty((B, T, C), dtype=np.float32)
    for b in range(B):
        out[b] = (res.results[2 * b]["out"].astype(np.float32)
                  + res.results[2 * b + 1]["out"].astype(np.float32) + bias)
    return out


# revision 5
# speedup vs baseline: 1.3164x; 1.0093x over previous
"""Causal self-attention (B=4, T=2048, C=1024, 16 heads) on 8 NeuronCores.

Sharding: core i handles batch b = i//2 and head-group hg = i%2 (8 heads each).
Each core computes the qkv projection for its heads, causal attention, and a
partial output projection (contraction over its 512 channels). The host sums
the two partial outputs per batch and adds b_proj + W_proj @ b_v (the v-bias
is never added on device: softmax rows sum to 1, so its effect on y is the
constant +b_v, which commutes with the projection).

Device kernel design (matmuls in bf16 with fp32 PSUM accumulation):
  - x^T resident in SBUF (packed on host so DMAs are contiguous); q,k are
    produced transposed ([head_dim, T] per head, two heads stacked per
    128-partition tile) so score blocks S^T = k^T.T @ q^T need no transposes.
  - softmax along the free axis without max subtraction (scores are small:
    |s|/8 <= ~0.5): one exp on ScalarE per j-step covering both heads (the
    two heads' score blocks live in one 2-bank PSUM tile); the causal
    triangle is masked by a 0/1 multiply on diagonal blocks only, and
    fully-masked column ranges are never computed.
  - v is produced naturally ([T, head_dim]); each head's PV lhsT is a
    128-column block [ones(64) | v(64)], so the PV matmul's output rows 0-63
    are the softmax denominator Z broadcast 64-fold -- same stream length as
    a 64-row output, i.e. the Z broadcast is free on the PE. The epilogue is
    then just reciprocal_approx_fast on rows 0-63 and one tensor multiply
    against rows 64-127, writing the normalized y straight to SBUF in bf16.
  - Emission interleaving: the attention phase is roughly balanced between
    ScalarE(exp) and the PE; projection work is chopped into small "filler"
    units emitted between attention j-steps so the Tile scheduler places
    them in the PE stream's dependency stalls.
  - Output is written bf16 in small per-queue chunks to cut the final DMA
    tail; the host accumulates the two partials in fp32.
"""

import numpy as np
import ml_dtypes

B, T, C = 4, 2048, 1024
NH, HS = 16, 64
NCORES = 8
LH = 8          # local heads per core
BF16 = ml_dtypes.bfloat16

_nc_cache = {}


def _build_nc():
    from collections import deque

    import concourse.bacc as bacc
    import concourse.tile as tile
    from concourse import mybir

    bf = mybir.dt.bfloat16
    f32 = mybir.dt.float32
    Exp = mybir.ActivationFunctionType.Exp

    nc = bacc.Bacc("TRN2", target_bir_lowering=False, debug=False, num_devices=NCORES)

    # packed layouts (host side): see _host_inputs
    xp = nc.dram_tensor("xp", [128, 16384], bf, kind="ExternalInput")
    wvp = nc.dram_tensor("wvp", [128, 4096], bf, kind="ExternalInput")
    wqkp = nc.dram_tensor("wqkp", [128, 8192], bf, kind="ExternalInput")
    wqkb2 = nc.dram_tensor("wqkb2", [128, 8], f32, kind="ExternalInput")
    wpp = nc.dram_tensor("wpp", [128, 4096], bf, kind="ExternalInput")
    tri = nc.dram_tensor("tri", [128, 128], bf, kind="ExternalInput")
    out = nc.dram_tensor("out", [T, C], bf, kind="ExternalOutput")

    with tile.TileContext(nc) as tc:
        with (
            tc.tile_pool(name="consts", bufs=1) as consts,
            tc.tile_pool(name="spool", bufs=2, space="PSUM") as spool,
            tc.tile_pool(name="ypool", bufs=2, space="PSUM") as ypool,
            tc.tile_pool(name="mpool", bufs=2, space="PSUM") as mpool,
            tc.tile_pool(name="ppool", bufs=4) as ppool,
            tc.tile_pool(name="opool", bufs=3) as opool,
            tc.tile_pool(name="rpool", bufs=4) as rpool,
        ):
            # x^T columns grouped quarter-major: col = 4096*q + 512*kb + tc
            xT_sb = consts.tile([128, 16384], bf)
            wv_sb = consts.tile([128, 4096], bf)
            wqk_sb = consts.tile([128, 8192], bf)
            wqkb2_sb = consts.tile([128, 8], f32)
            wp_sb = consts.tile([128, 4096], bf)
            tri_sb = consts.tile([128, 128], bf)
            qk_sb = consts.tile([128, 16384], bf)
            # per (t, head): 128 cols = [ones(64) | v(64)]
            v_sb = consts.tile([128, 16, 8, 128], bf)
            yp_sb = [consts.tile([128, T], bf, name=f"yp_sb{g}") for g in range(4)]

            # ones columns for the Z-broadcast rows, off the critical engines
            for t in range(16):
                nc.gpsimd.memset(v_sb[:, t, :, 0:64], 1.0)

            # ---- input DMAs, interleaved per contraction chunk kb so the
            # qk/v preamble matmuls can start as soon as chunk 0 lands
            for kb in range(8):
                nc.sync.dma_start(out=xT_sb[:, 512 * kb : 512 * kb + 512],
                                  in_=xp[:, 512 * kb : 512 * kb + 512])
                nc.sync.dma_start(out=wqk_sb[:, 1024 * kb : 1024 * kb + 512],
                                  in_=wqkp[:, 1024 * kb : 1024 * kb + 512])
                nc.sync.dma_start(out=wqk_sb[:, 1024 * kb + 512 : 1024 * kb + 1024],
                                  in_=wqkp[:, 1024 * kb + 512 : 1024 * kb + 1024])
                nc.sync.dma_start(out=wv_sb[:, 512 * kb : 512 * kb + 512],
                                  in_=wvp[:, 512 * kb : 512 * kb + 512])
            nc.sync.dma_start(out=wqkb2_sb[:, :], in_=wqkb2[:, :])
            nc.sync.dma_start(out=tri_sb[:, :], in_=tri[:, :])
            for i in range(12):
                nc.sync.dma_start(out=xT_sb[:, 4096 + 1024 * i : 4096 + 1024 * i + 1024],
                                  in_=xp[:, 4096 + 1024 * i : 4096 + 1024 * i + 1024])
            for i in range(4):
                nc.sync.dma_start(out=wp_sb[:, 1024 * i : 1024 * i + 1024],
                                  in_=wpp[:, 1024 * i : 1024 * i + 1024])

            def xslice(kb, t):
                """x^T block for contraction chunk kb, T-tile t (128 cols)."""
                o = 4096 * (t // 4) + 512 * kb + 128 * (t % 4)
                return xT_sb[:, o : o + 128]

            # ---- emission units -------------------------------------------
            def v_group(t):
                """v projection for T-tile t, all 8 heads (512 cols)."""
                units = []
                ps_box = []

                def mk_mm(kb):
                    def f():
                        if kb == 0:
                            ps_box.append(mpool.tile([128, 8, 64], f32, tag="mm", name="vps"))
                        nc.tensor.matmul(ps_box[0][:, :, :], xslice(kb, t),
                                         wv_sb[:, 512 * kb : 512 * kb + 512],
                                         start=(kb == 0), stop=(kb == 7))
                    return f

                for kb in range(8):
                    units.append(mk_mm(kb))

                def epi():
                    nc.vector.tensor_copy(v_sb[:, t, :, 64:128], ps_box[0][:, :, :])
                units.append(epi)
                return units

            def qk_group(o, c):
                """q/k projection chunk: o-tile o (0-3 q, 4-7 k), T-chunk c."""
                units = []
                ps_box = []

                def mk_mm(kb):
                    def f():
                        if kb == 0:
                            ps_box.append(mpool.tile([128, 512], f32, tag="mm", name="qkps"))
                        nc.tensor.matmul(
                            ps_box[0][:, :],
                            wqk_sb[:, 1024 * kb + 128 * o : 1024 * kb + 128 * o + 128],
                            xT_sb[:, 4096 * c + 512 * kb : 4096 * c + 512 * kb + 512],
                            start=(kb == 0), stop=(kb == 7))
                    return f

                for kb in range(8):
                    units.append(mk_mm(kb))

                def epi():
                    nc.vector.tensor_scalar_add(
                        qk_sb[:, 2048 * o + 512 * c : 2048 * o + 512 * c + 512],
                        ps_box[0][:, :],
                        wqkb2_sb[:, o : o + 1])
                units.append(epi)
                return units

            # ---- preamble: v tiles 0-3, qk chunk 0 for pair 0
            for o in (0, 4):
                for u in qk_group(o, 0):
                    u()
            for t in range(4):
                for u in v_group(t):
                    u()

            # ---- filler queue: remaining projections in data-dependency
            # order; emitted piecewise between attention steps. ensure()
            # force-drains the queue up to a required producer.
            fillers = deque()
            emitted = set()
            emitted.update({("v", t) for t in range(4)})
            emitted.add(("qk", 0, 0))
            emitted.add(("qk", 4, 0))

            def q_v(t):
                fillers.append((("v", t), v_group(t)))

            def q_qk(o, c):
                fillers.append((("qk", o, c), qk_group(o, c)))

            def out_group(t, o2):
                """output-projection chunk: T-tile t, column half o2."""
                units = []
                ps_box = []

                def mk_mm(g):
                    def f():
                        if g == 0:
                            ps_box.append(mpool.tile([128, 512], f32, tag="mm", name="fps"))
                        nc.tensor.matmul(
                            ps_box[0][:, :],
                            yp_sb[g][:, 128 * t : 128 * t + 128],
                            wp_sb[:, 1024 * g + 512 * o2 : 1024 * g + 512 * o2 + 512],
                            start=(g == 0), stop=(g == 3))
                    return f

                for g in range(4):
                    units.append(mk_mm(g))

                def epi():
                    ot = opool.tile([128, 512], bf, tag="o", name="ot")
                    nc.vector.tensor_copy(ot[:, :], ps_box[0][:, :])
                    for r in range(2):
                        nc.sync.dma_start(
                            out=out[128 * t + 64 * r : 128 * t + 64 * r + 64,
                                    512 * o2 : 512 * o2 + 512],
                            in_=ot[64 * r : 64 * r + 64, :])
                units.append(epi)
                return units

            # g=0 tail chunks + v(.) just ahead of their use
            for c in range(1, 4):
                q_qk(0, c)
                q_qk(4, c)
                for t in range(4 * c, 4 * c + 4):
                    q_v(t)
            # g=1..3
            for g in range(1, 4):
                for c in range(4):
                    q_qk(g, c)
                    q_qk(4 + g, c)

            pending = deque()  # units of the partially-emitted current group
            inflight = [None]   # key of the group in `pending`

            def _step():
                pending.popleft()()
                if not pending:
                    emitted.add(inflight[0])
                    inflight[0] = None

            def pump(n):
                for _ in range(n):
                    if pending:
                        _step()
                    elif fillers:
                        key, units = fillers.popleft()
                        inflight[0] = key
                        pending.extend(units)
                        _step()

            def ensure(key):
                if key in emitted:
                    return
                while pending:
                    _step()
                while key not in emitted:
                    k2, units = fillers.popleft()
                    inflight[0] = k2
                    pending.extend(units)
                    while pending:
                        _step()

            # ---- attention ------------------------------------------------
            for g in range(4):
                for c in range(4):
                    ensure(("qk", g, c))
                    ensure(("qk", 4 + g, c))
                    y01 = [ypool.tile([128, 512], f32, tag="y", name=f"yzaug{h}") for h in range(2)]
                    nj = 4 * c + 4
                    for j in range(nj):
                        ensure(("qk", 4 + g, j // 4))
                        ensure(("v", j))
                        d = max(0, j - 4 * c)
                        w = 512 - 128 * d
                        koff = 2048 * (4 + g) + 128 * j
                        qoff = 2048 * g + 512 * c + 128 * d
                        S = spool.tile([128, 2, 512], f32, tag="s", name="S")
                        for h in range(2):
                            lo, hi = 64 * h, 64 * h + 64
                            nc.tensor.matmul(S[:, h, 0:w],
                                             qk_sb[lo:hi, koff : koff + 128],
                                             qk_sb[lo:hi, qoff : qoff + w],
                                             start=True, stop=True)
                        pT = ppool.tile([128, 2, 512], bf, tag="p", name="pT")
                        nc.scalar.activation(pT[:, :, 128 * d : 512], S[:, :, 0:w],
                                             Exp, scale=0.125)
                        if j >= 4 * c:
                            for h in range(2):
                                nc.vector.tensor_mul(pT[:, h, 128 * d : 128 * d + 128],
                                                     pT[:, h, 128 * d : 128 * d + 128],
                                                     tri_sb[:, :])
                        for h in range(2):
                            nc.tensor.matmul(
                                y01[h][:, 128 * d : 512],
                                v_sb[:, j, 2 * g + h, :],
                                pT[:, h, 128 * d : 512],
                                start=(j == 0), stop=(j == nj - 1))
                        pump(2)

                    # epilogue: rows 0-63 of y01 hold Z broadcast 64-fold;
                    # reciprocal + one multiply yields normalized y in bf16
                    for h in range(2):
                        rz = rpool.tile([64, 512], f32, tag="r", name="rz")
                        nc.vector.reciprocal_approx_fast(rz[:, :], y01[h][0:64, :])
                        nc.vector.tensor_mul(
                            yp_sb[g][64 * h : 64 * h + 64, 512 * c : 512 * c + 512],
                            y01[h][64:128, :],
                            rz[:, :])
                    pump(6)
                    # once g=3's c-chunk is done, the output-projection
                    # T-tiles it completes become filler work that overlaps
                    # the rest of the attention phase
                    if g == 3:
                        for t in range(4 * c, 4 * c + 4):
                            for o2 in range(2):
                                fillers.append(
                                    (("out", t, o2), out_group(t, o2)))

            while pending or fillers:
                pump(1)

    nc.compile()
    return nc


def _host_inputs(x, W_qkv, b_qkv):
    """Per-core input tensors (host-side slicing/transpose/pack/cast)."""
    xps = []
    for b in range(B):
        xt = np.ascontiguousarray(x[b].T).astype(BF16)          # (C, T)
        # col = 4096*q + 512*kb + tc
        xps.append(np.ascontiguousarray(
            xt.reshape(8, 128, 4, 512).transpose(1, 2, 0, 3).reshape(128, 16384)))

    wqks, wqkb2s, wvs = [], [], []
    for hg in range(2):
        qs, ks, vs = 512 * hg, 1024 + 512 * hg, 2048 + 512 * hg
        wqkm = np.empty((C, 1024), dtype=BF16)
        wqkm[:, 0:512] = W_qkv[qs : qs + 512].T.astype(BF16)
        wqkm[:, 512:1024] = W_qkv[ks : ks + 512].T.astype(BF16)
        wqks.append(np.ascontiguousarray(
            wqkm.reshape(8, 128, 1024).transpose(1, 0, 2).reshape(128, 8192)))
        bqk = np.concatenate([b_qkv[qs : qs + 512], b_qkv[ks : ks + 512]])
        wqkb2s.append(np.ascontiguousarray(bqk.reshape(8, 128).T).astype(np.float32))

        wvm = W_qkv[vs : vs + 512].T.astype(BF16)               # (C, 512)
        wvs.append(np.ascontiguousarray(
            wvm.reshape(8, 128, 512).transpose(1, 0, 2).reshape(128, 4096)))

    tri = np.triu(np.ones((128, 128))).astype(BF16)
    return xps, wqks, wqkb2s, wvs, tri


def kernel(x, W_qkv, b_qkv, W_proj, b_proj):
    from concourse.bass_utils import run_bass_kernel_spmd

    if "nc" not in _nc_cache:
        _nc_cache["nc"] = _build_nc()
    nc = _nc_cache["nc"]

    x = np.asarray(x, dtype=np.float32)
    W_qkv = np.asarray(W_qkv, dtype=np.float32)
    b_qkv = np.asarray(b_qkv, dtype=np.float32)
    W_proj = np.asarray(W_proj, dtype=np.float32)
    b_proj = np.asarray(b_proj, dtype=np.float32)

    xps, wqks, wqkb2s, wvs, tri = _host_inputs(x, W_qkv, b_qkv)
    wps = []
    for hg in range(2):
        wpm = np.ascontiguousarray(W_proj[:, 512 * hg : 512 * hg + 512].T).astype(BF16)
        wps.append(np.ascontiguousarray(
            wpm.reshape(4, 128, 1024).transpose(1, 0, 2).reshape(128, 4096)))

    in_maps = []
    for core in range(NCORES):
        b, hg = core // 2, core % 2
        in_maps.append({
            "xp": xps[b],
            "wqkp": wqks[hg],
            "wqkb2": wqkb2s[hg],
            "wvp": wvs[hg],
            "wpp": wps[hg],
            "tri": tri,
        })

    try:
        res = run_bass_kernel_spmd(nc, in_maps, core_ids=list(range(NCORES)),
                                   **_nc_cache.get("run_kwargs", {}))
    except Exception:
        # transient axon-terminal device errors recover on retry
        import time as _time
        _time.sleep(2.0)
        res = run_bass_kernel_spmd(nc, in_maps, core_ids=list(range(NCORES)),
                                   **_nc_cache.get("run_kwargs", {}))
    _nc_cache["last_result"] = res

    # v-bias never reaches the device: y = P(v+b)/Z = Pv/Z + b_v, so its
    # projected effect is the constant W_proj @ b_v, folded in here.
    bias = b_proj + W_proj @ b_qkv[2 * C : 3 * C]
    out = np.empty((B, T, C), dtype=np.float32)
    for b in range(B):
        out[b] = (res.results[2 * b]["out"].astype(np.float32)
                  + res.results[2 * b + 1]["out"].astype(np.float32) + bias)
    return out
